# revision 24
# baseline (speedup 1.0000x reference)
"""Single-head causal self-attention on 8 NeuronCores (data-parallel over batch).

Reference computation (per batch element b):
    Q = X @ Wq + bq; K = X @ Wk + bk; V = X @ Wv + bv        # [T, DK]
    S = Q @ K.T / sqrt(DK)  (causal masked)
    out = softmax(S) @ V                                      # [T, DK]

Device strategy (one batch element per core):
  - Host passes X.T [C, T] in bf16 so every DMA row is contiguous and
    half the bytes of fp32 (X DMA paces the projection phase).
  - Two full-width projection passes with packed stationaries:
      pass A: [Wv | Wk] -> psum rows 0:64 = V.T, rows 64:128 = K.T
      pass B: [Wq | Wq] -> Q.T duplicated in both partition halves
    Biases are added exactly during the PSUM->SBUF drain; pass-A drains
    on ScalarE (activation Identity + bias vec), pass-B on VectorE, both
    interleaved into the last k-tile's matmuls so the tail is one drain.
  - V.T is PE-transposed into natural [s, dk] tiles with a ones column
    appended; the ones column makes the output matmul also produce the
    softmax denominator l (row 64 of the output).
  - Scores are computed transposed: S.T[s, t] = K.T^T @ Q.T, so softmax's
    exp (ScalarE, scale=1/8 fused) and the P@V contraction (over s = the
    partition dim) need no large transposes. Causality = skip tiles below
    the diagonal + one upper-triangular 128x128 mask multiply per s-tile.
  - All matmul operands are bf16 (fp32 PSUM accumulation); fp32 tensors
    remain only in PSUM and the final output.
  - Device output per core: [65, T] = rows 0:64 unnormalized O.T, row 64 l.
    Host computes (O_unnorm / l).T in fp32.
"""

import sys

sys.path.insert(0, "/opt/trn_rl_repo")

import numpy as np
import ml_dtypes

B, T, C, DK = 8, 2048, 1024, 64
KT = C // 128          # 8 k-tiles in the contraction over C
NS = T // 128          # 16 s-tiles (key blocks)
NCHUNK = T // 512      # 4 output chunks of 512
SCALE = 1.0 / np.sqrt(DK)
BF16 = np.dtype(ml_dtypes.bfloat16)

_CACHE = {}


def _build():
    from concourse import bass, bacc, tile

    mybir = bass.mybir
    f32 = mybir.dt.float32
    bf16 = mybir.dt.bfloat16

    nc = bacc.Bacc(
        "TRN2", target_bir_lowering=False, debug=False, num_devices=B
    )

    xt_d = nc.dram_tensor("xt", [KT, 128, T], bf16, kind="ExternalInput")
    wvk_d = nc.dram_tensor("wvk", [128, KT * 128], bf16, kind="ExternalInput")
    wqq_d = nc.dram_tensor("wqq", [128, KT * 128], bf16, kind="ExternalInput")
    bvk_d = nc.dram_tensor("bvk", [128, 1], f32, kind="ExternalInput")
    bqq_d = nc.dram_tensor("bqq", [128, 1], f32, kind="ExternalInput")
    out_d = nc.dram_tensor("out", [65, T], f32, kind="ExternalOutput")

    # one packed const block: cols 0:128 tri-mask, 128:192 ident (rows 0:64)
    cst_np = np.zeros((128, 192), dtype=BF16)
    cst_np[:, 0:128] = np.triu(np.ones((128, 128), dtype=np.float32)).astype(BF16)
    cst_np[0:64, 128:192] = np.eye(64, dtype=np.float32).astype(BF16)
    cst_d = nc.inline_tensor(cst_np, "cst")

    EXP = mybir.ActivationFunctionType.Exp

    with tile.TileContext(nc) as tc:
        with tc.tile_pool(name="const", bufs=1) as cpool, \
             tc.tile_pool(name="weights", bufs=1) as wpool, \
             tc.tile_pool(name="x", bufs=1) as xpool, \
             tc.tile_pool(name="acts", bufs=1) as apool:

            # small consts + weights first so their DMAs clear the queues
            # before the X stream
            cst = cpool.tile([128, 192], bf16)
            nc.gpsimd.dma_start(out=cst[:], in_=cst_d[:])
            tri = cst[:, 0:128]
            ident = cst[0:64, 128:192]
            bvk = cpool.tile([128, 1], f32)
            nc.gpsimd.dma_start(out=bvk[:], in_=bvk_d[:])
            bqq = cpool.tile([128, 1], f32)
            nc.gpsimd.dma_start(out=bqq[:], in_=bqq_d[:])

            wvk = wpool.tile([128, KT * 128], bf16)
            wqq = wpool.tile([128, KT * 128], bf16)
            nc.scalar.dma_start(out=wvk[:], in_=wvk_d[:])
            nc.sync.dma_start(out=wqq[:], in_=wqq_d[:])

            dma_engs = [nc.sync, nc.gpsimd, nc.scalar]
            xts = []
            for k in range(KT):
                xk = xpool.tile([128, T], bf16, tag=f"x{k}")
                dma_engs[k % 3].dma_start(out=xk[:], in_=xt_d[k])
                xts.append(xk)

            # persistent activations
            vk = apool.tile([128, T], bf16, tag="vk")    # V.T rows 0:64, K.T rows 64:128
            qq = apool.tile([128, T], bf16, tag="qq")    # Q.T in both halves
            v1 = apool.tile([128, NS * 65], bf16, tag="v1")  # [V_i | 1] stationaries
            osb = apool.tile([65, T], f32, tag="osb")

            nc.gpsimd.memset(v1[:], 1.0)

            # ---------------- projections ----------------
            with tc.tile_pool(name="pproj", bufs=1, space="PSUM") as pproj:
                psA = pproj.tile([128, T], f32, tag="psA")
                psB = pproj.tile([128, T], f32, tag="psB")
                for k in range(KT):
                    last = k == KT - 1
                    for c in range(NCHUNK):
                        sl = slice(512 * c, 512 * (c + 1))
                        nc.tensor.matmul(
                            psA[:, sl],
                            wvk[:, 128 * k:128 * (k + 1)],
                            xts[k][:, sl],
                            start=(k == 0), stop=last,
                        )
                        if last:
                            # exact bias add during drain, on ScalarE
                            nc.scalar.add(vk[:, sl], psA[:, sl], bvk[:])
                    for c in range(NCHUNK):
                        sl = slice(512 * c, 512 * (c + 1))
                        nc.tensor.matmul(
                            psB[:, sl],
                            wqq[:, 128 * k:128 * (k + 1)],
                            xts[k][:, sl],
                            start=(k == 0), stop=last,
                        )
                        if last:
                            nc.vector.tensor_scalar_add(qq[:, sl], psB[:, sl], bqq[:])

            # ---------------- V transposes ----------------
            with tc.tile_pool(name="pv", bufs=2, space="PSUM") as pv:
                for i in range(NS):
                    vt = pv.tile([128, 64], bf16, tag="vt")
                    nc.tensor.transpose(
                        vt[:], vk[0:64, 128 * i:128 * (i + 1)], ident[:]
                    )
                    nc.vector.tensor_copy(v1[:, 65 * i:65 * i + 64], vt[:])

            # ---------------- attention ----------------
            with tc.tile_pool(name="po", bufs=1, space="PSUM") as po, \
                 tc.tile_pool(name="pst", bufs=2, space="PSUM") as pst, \
                 tc.tile_pool(name="et", bufs=3) as etpool:

                ops = [
                    po.tile([65, 512], f32, tag=f"o{j}", name=f"o{j}")
                    for j in range(NCHUNK)
                ]

                for i in range(NS):
                    ts = 128 * i
                    jmin = i // 4
                    et = etpool.tile([128, T], bf16, tag="et")
                    if ts > 512 * jmin:
                        nc.gpsimd.memset(et[:, 512 * jmin:ts], 0.0)
                    for tb in range(ts // 1024, 2):
                        st = pst.tile([128, 1024], f32, tag="st")
                        for cc in range(2):
                            t0 = 1024 * tb + 512 * cc
                            t1 = t0 + 512
                            if t1 <= ts:
                                continue
                            lo = max(t0, ts)  # exact 128-aligned diagonal start
                            nc.tensor.matmul(
                                st[:, lo - 1024 * tb:t1 - 1024 * tb],
                                vk[64:128, 128 * i:128 * (i + 1)],
                                qq[64:128, lo:t1],
                                start=True, stop=True,
                            )
                        off = max(0, ts - 1024 * tb)
                        nc.scalar.activation(
                            et[:, 1024 * tb + off:1024 * (tb + 1)],
                            st[:, off:1024],
                            EXP, scale=SCALE,
                        )
                    # causal mask on the diagonal 128-block
                    nc.vector.tensor_mul(
                        et[:, ts:ts + 128], et[:, ts:ts + 128], tri[:]
                    )
                    for j in range(jmin, NCHUNK):
                        lo = max(512 * j, ts)
                        nc.tensor.matmul(
                            ops[j][:, lo - 512 * j:512],
                            v1[:, 65 * i:65 * i + 65],
                            et[:, lo:512 * (j + 1)],
                            start=(i == 0), stop=(i == 4 * j + 3),
                        )
                    # drain any output chunk whose accumulation just finished
                    for j in range(jmin, NCHUNK):
                        if i == 4 * j + 3:
                            sl = slice(512 * j, 512 * (j + 1))
                            nc.vector.tensor_copy(osb[:, sl], ops[j][:])
                            nc.sync.dma_start(out=out_d[:, sl], in_=osb[:, sl])

    nc.compile()
    return nc


def _get_nc():
    if "nc" not in _CACHE:
        _CACHE["nc"] = _build()
    return _CACHE["nc"]


def make_in_maps(X, Wq, bq, Wk, bk, Wv, bv):
    X = np.asarray(X, dtype=np.float32)
    Wq = np.asarray(Wq, dtype=np.float32)
    Wk = np.asarray(Wk, dtype=np.float32)
    Wv = np.asarray(Wv, dtype=np.float32)
    bq = np.asarray(bq, dtype=np.float32)
    bk = np.asarray(bk, dtype=np.float32)
    bv = np.asarray(bv, dtype=np.float32)

    wvk = np.ascontiguousarray(
        np.concatenate([Wv, Wk], axis=1).reshape(KT, 128, 128)
        .transpose(1, 0, 2).reshape(128, KT * 128)
    ).astype(BF16)
    wqq = np.ascontiguousarray(
        np.concatenate([Wq, Wq], axis=1).reshape(KT, 128, 128)
        .transpose(1, 0, 2).reshape(128, KT * 128)
    ).astype(BF16)
    bvk = np.concatenate([bv, bk]).reshape(128, 1).astype(np.float32)
    bqq = np.concatenate([bq, bq]).reshape(128, 1).astype(np.float32)

    in_maps = []
    for b in range(B):
        xt = np.ascontiguousarray(X[b].T.astype(BF16)).reshape(KT, 128, T)
        in_maps.append(
            {"xt": xt, "wvk": wvk, "wqq": wqq, "bvk": bvk, "bqq": bqq}
        )
    return in_maps


def kernel(X, Wq, bq, Wk, bk, Wv, bv):
    from concourse.bass_utils import run_bass_kernel_spmd

    nc = _get_nc()
    in_maps = make_in_maps(X, Wq, bq, Wk, bk, Wv, bv)
    res = run_bass_kernel_spmd(nc, in_maps, list(range(B)))

    out = np.empty((B, T, DK), dtype=np.float32)
    for b in range(B):
        r = res.results[b]["out"]
        out[b] = (r[:64] / r[64:65]).T
    return out
